# revision 25
# baseline (speedup 1.0000x reference)
"""Multi-head attention (B=2, S=2048, E=1024, H=16, causal) on 8 trn2 NeuronCores.

Sharding: 8 cores = 2 batches x 4 head-groups (4 heads / 256 embed dims per core).
Host sums the 4 partial output projections per batch and stacks batches.

v3 design:
  - Host pre-tiles every input into the exact [.., 128, W] tile layout so each
    dma_start is one contiguous DRAM read; DMAs are emitted in first-use order
    (wk -> xk[0] -> wq -> xq[0] -> ...) so the PE starts within a few us.
  - Scores via row-tiled K=64 matmul pairs: heads 2i/2i+1 packed on partitions
    0:64 / 64:128 of one kT/qT tile; the two matmuls occupy disjoint PE row
    groups and execute concurrently (~176ns per pair vs 227ns for one K=128
    matmul, HW-measured).
  - Causal mask: exp runs unmasked, then one DVE multiply zeroes the upper
    triangle of the 128-wide diagonal strip of P (replaces 64 mask matmuls).
  - One exp per (pair, kt): ACT reads the 2-bank PSUM pair tile with a 3D AP
    and writes both heads' P tiles in one call.
  - Precision: scores/AV in f32r (raw fp32 bits, PE rounds to ~2^-11); the
    value and output projections run as 3-term bf16 hi/lo splits
       v   = xv_h@wv_h + xv_l@wv_h + xv_h@wv_l     (all bf16, ~2^-17 effective)
       out = ctx_h@wo_h + ctx_l@wo_h + ctx_h@wo_l
    which beats f32r single-pass precision AND runs ~25% faster per matmul.
  - All copies on DVE (vector); ACT does exp only.
  - Projection work for chunk rc=QT+1 and the output projection of QT-1 are
    emitted round-robin in small quanta between attention kt-steps of QT so
    the PE never idles while ACT runs exp (keeps HAM warm).
"""

import sys

if "/opt/trn_rl_repo" not in sys.path:
    sys.path.insert(0, "/opt/trn_rl_repo")

import numpy as np

B = 2
S = 2048
E = 1024
H = 16
D = 64
N_CORES = 8
GROUPS = 4              # head-groups (cores per batch)
GH = H // GROUPS        # heads per core = 4
GD = GH * D             # qkv dims per core = 256
QTW = 512               # query-tile width
QTN = S // QTW          # 4
KTN = S // 128          # 16
ECN = E // 128          # embed chunks = 8
VSEG = GH * 128         # v_aug cols per kt = 512 (128-col aligned segments)

_cache: dict = {}


def _emit(nc, tc, tile, mybir, causal):
    f32 = mybir.dt.float32
    f32r = mybir.dt.float32r
    bf16 = mybir.dt.bfloat16
    f16 = mybir.dt.float16
    Exp = mybir.ActivationFunctionType.Exp
    inv_sqrt_e = 1.0 / float(np.sqrt(E))

    xq4 = nc.dram_tensor("xq4", [ECN, QTN, 128, QTW], f16, kind="ExternalInput").ap()
    xk4 = nc.dram_tensor("xk4", [ECN, QTN, 128, QTW], f16, kind="ExternalInput").ap()
    xvh4 = nc.dram_tensor("xvh4", [ECN, QTN, 128, QTW], bf16, kind="ExternalInput").ap()
    xvl4 = nc.dram_tensor("xvl4", [ECN, QTN, 128, QTW], bf16, kind="ExternalInput").ap()
    wq4 = nc.dram_tensor("wq4", [ECN, 128, GD], f16, kind="ExternalInput").ap()
    wk4 = nc.dram_tensor("wk4", [ECN, 128, GD], f16, kind="ExternalInput").ap()
    wvh4 = nc.dram_tensor("wvh4", [ECN, 128, GD], bf16, kind="ExternalInput").ap()
    wvl4 = nc.dram_tensor("wvl4", [ECN, 128, GD], bf16, kind="ExternalInput").ap()
    woh = nc.dram_tensor("woh", [2, 128, E], bf16, kind="ExternalInput").ap()
    wol = nc.dram_tensor("wol", [2, 128, E], bf16, kind="ExternalInput").ap()
    vones = nc.dram_tensor("vones", [128, KTN * GH], f32, kind="ExternalInput").ap()
    bones2 = nc.dram_tensor("bones2", [2, 128], f32r, kind="ExternalInput").ap()
    if causal:
        tri01 = nc.dram_tensor("tri01", [128, 128], f32, kind="ExternalInput").ap()
    out = nc.dram_tensor("out", [S, E], f32, kind="ExternalOutput").ap()
    dbg = nc.dram_tensor("dbg", [1, 8], f32, kind="ExternalOutput").ap()

    with (
        tc.tile_pool(name="xp", bufs=12) as xp,       # f32r x chunks (K/Q)
        tc.tile_pool(name="xhp", bufs=8) as xhp,      # xv hi chunks (bf16)
        tc.tile_pool(name="xlp", bufs=8) as xlp,      # xv lo chunks (bf16)
        tc.tile_pool(name="qpp", bufs=2) as qpp,      # per-QT q pair tiles
        tc.tile_pool(name="persist", bufs=1) as pp,
        tc.tile_pool(name="ptp", bufs=6) as ptp,
        tc.tile_pool(name="ctxp", bufs=3) as ctxp,    # per-QT ctx hi/lo (bf16)
        tc.tile_pool(name="normp", bufs=2) as normp,
        tc.tile_pool(name="bcp", bufs=2) as bcp,
        tc.tile_pool(name="ostp", bufs=4) as ostp,
        tc.tile_pool(name="ps_mm", bufs=2, space="PSUM") as ps_mm,
        tc.tile_pool(name="ps_sT", bufs=2, space="PSUM") as ps_sT,
        tc.tile_pool(name="ps_ctx", bufs=2, space="PSUM") as ps_ctx,
    ):
        # ---------------- persistent tiles ----------------
        kp = [pp.tile([128, S], f32r, tag=f"kp{i}", name=f"kp{i}") for i in range(2)]
        v_aug = pp.tile([128, KTN * VSEG], f32r, tag="v_aug")
        wo_hi = [pp.tile([128, E], bf16, tag=f"woh{i}", name=f"woh{i}") for i in range(2)]
        wo_lo = [pp.tile([128, E], bf16, tag=f"wol{i}", name=f"wol{i}") for i in range(2)]
        wk = [pp.tile([128, GD], f16, tag=f"wk{i}", name=f"wk{i}") for i in range(ECN)]
        wq = [pp.tile([128, GD], f16, tag=f"wq{i}", name=f"wq{i}") for i in range(ECN)]
        wvh = [pp.tile([128, GD], bf16, tag=f"wvh{i}", name=f"wvh{i}") for i in range(ECN)]
        wvl = [pp.tile([128, GD], bf16, tag=f"wvl{i}", name=f"wvl{i}") for i in range(ECN)]
        bones_sb = pp.tile([2, 128], f32r, tag="bones_sb")
        vones_sb = pp.tile([128, KTN * GH], f32, tag="vones_sb")
        if causal:
            tri01_sb = pp.tile([128, 128], f32, tag="tri01_sb")

        qp_tiles: dict = {}

        # Dummy warmup accumulation: keeps the PE HAM clock at 8/8 through the
        # DMA-bound first chunk (uses a score-pool PSUM slot, idle until QT0).
        warm = {"ps": None, "n": 0}

        def emit_warm(k=2):
            if warm["ps"] is None:
                warm["ps"] = ps_sT.tile([128, 2 * QTW], f32, tag="sT", name="warm")
            for _ in range(k):
                nc.tensor.matmul(
                    warm["ps"][:, 0:GD],
                    wk[0][:, 0:128],
                    wk[0][:],
                    start=(warm["n"] == 0),
                    stop=False,
                    skip_group_check=True,
                )
                warm["n"] += 1

        def finish_warm():
            if warm["ps"] is None:
                return
            nc.tensor.matmul(
                warm["ps"][:, 0:GD],
                wk[0][:, 0:128],
                wk[0][:],
                start=False,
                stop=True,
                skip_group_check=True,
            )
            wsb = normp.tile([1, 8], f32, tag="wsb")
            nc.vector.tensor_copy(wsb[:], warm["ps"][0:1, 0:8])
            nc.sync.dma_start(dbg[:], wsb[:])
            warm["ps"] = None

        def with_warm(gen):
            for x in gen:
                emit_warm(2)
                yield x

        # ---------------- projection emission (quantum generator) -----
        def proj_quanta(rc):
            r_sl = slice(QTW * rc, QTW * (rc + 1))
            qcur = [
                qpp.tile([128, QTW], f32r, tag=f"qp{i}", name=f"qp{rc}_{i}")
                for i in range(2)
            ]
            qp_tiles[rc] = qcur
            for which, xsrc, wts, w4 in (("k", xk4, wk, wk4), ("q", xq4, wq, wq4)):
                if rc == 0:
                    for ec in range(ECN):
                        nc.sync.dma_start(wts[ec][:], w4[ec])
                xsl = []
                for ec in range(ECN):
                    t = xp.tile([128, QTW], f16, tag="x", name=f"x{which}{rc}_{ec}")
                    nc.sync.dma_start(t[:], xsrc[ec, rc])
                    xsl.append(t)
                    if ec % 4 == 3:
                        yield
                for dt_ in range(2):
                    ps = ps_mm.tile([128, QTW], f32, tag="mm")
                    for ec in range(ECN):
                        nc.tensor.matmul(
                            ps[:],
                            wts[ec][:, 128 * dt_ : 128 * (dt_ + 1)],
                            xsl[ec][:],
                            start=(ec == 0),
                            stop=(ec == ECN - 1),
                        )
                        if ec % 4 == 3:
                            yield
                    if which == "k":
                        nc.vector.tensor_copy(kp[dt_][:, r_sl], ps[:])
                    else:
                        nc.vector.tensor_copy(qcur[dt_][:], ps[:])
                    yield
            # V: 3-term bf16 projection from host-split hi/lo
            if rc == 0:
                for ec in range(ECN):
                    nc.sync.dma_start(wvh[ec][:], wvh4[ec])
                    nc.sync.dma_start(wvl[ec][:], wvl4[ec])
            xh, xl = [], []
            for ec in range(ECN):
                th = xhp.tile([128, QTW], bf16, tag="xh", name=f"xvh{rc}_{ec}")
                tl = xlp.tile([128, QTW], bf16, tag="xl", name=f"xvl{rc}_{ec}")
                nc.sync.dma_start(th[:], xvh4[ec, rc])
                nc.sync.dma_start(tl[:], xvl4[ec, rc])
                xh.append(th)
                xl.append(tl)
                if ec % 4 == 3:
                    yield
            if rc == 0:
                # consts needed by attention QT=0 (emitted after first-use DMAs)
                nc.sync.dma_start(vones_sb[:], vones[:])
                if causal:
                    nc.sync.dma_start(tri01_sb[:], tri01[:])
                vh_view = v_aug[:].rearrange("p (k h d) -> p k h d", k=KTN, h=GH)
                nc.vector.tensor_copy(
                    vh_view[:, :, :, D],
                    vones_sb[:].rearrange("p (k h) -> p k h", k=KTN),
                )
            for rt in range(4):
                kt = rc * 4 + rt
                c_sl = slice(128 * rt, 128 * (rt + 1))
                ps = ps_mm.tile([128, GD], f32, tag="mm")
                n3 = 3 * ECN
                i3 = 0
                for ec in range(ECN):
                    for sta, mov in ((xh[ec], wvh[ec]), (xl[ec], wvh[ec]), (xh[ec], wvl[ec])):
                        nc.tensor.matmul(
                            ps[:],
                            sta[:, c_sl],
                            mov[:],
                            start=(i3 == 0),
                            stop=(i3 == n3 - 1),
                        )
                        i3 += 1
                    if ec % 2 == 1:
                        yield
                for h in range(GH):
                    nc.vector.tensor_copy(
                        v_aug[:, VSEG * kt + 128 * h : VSEG * kt + 128 * h + D],
                        ps[:, D * h : D * (h + 1)],
                    )
                yield

        # ---------------- output projection (quantum generator) -------
        def outproj_quanta(QT, cb, cl):
            for rt in range(4):
                r0 = QTW * QT + 128 * rt
                c_sl = slice(128 * rt, 128 * (rt + 1))
                ost = ostp.tile([128, E], f32, tag="ost")
                for ct in range(2):
                    po_ = ps_mm.tile([128, QTW], f32, tag="mm")
                    w_sl = slice(QTW * ct, QTW * (ct + 1))
                    i6 = 0
                    for i in range(2):
                        for sta, mov in (
                            (cb[i], wo_hi[i]),
                            (cl[i], wo_hi[i]),
                            (cb[i], wo_lo[i]),
                        ):
                            nc.tensor.matmul(
                                po_[:],
                                sta[:, c_sl],
                                mov[:, w_sl],
                                start=(i6 == 0),
                                stop=(i6 == 5),
                            )
                            i6 += 1
                        yield
                    nc.vector.tensor_copy(ost[:, w_sl], po_[:])
                nc.sync.dma_start(out[r0 : r0 + 128, :], ost[:])
                yield

        def drain(gen):
            if gen is not None:
                for _ in gen:
                    pass

        def pump(gen, n=1):
            if gen is None:
                return None
            try:
                for _ in range(n):
                    next(gen)
                return gen
            except StopIteration:
                return None

        def rr(*gens):
            """Round-robin interleave of generators."""
            gens = [g for g in gens if g is not None]
            while gens:
                for g in list(gens):
                    try:
                        yield next(g)
                    except StopIteration:
                        gens.remove(g)

        # preload the ACT exp table set (~2.7us) before it's needed at QT0:
        # a dummy exp on the first-loaded const forces the PSEUDO_LOAD early.
        nc.sync.dma_start(bones_sb[:], bones2[:])
        scr = normp.tile([2, 128], f32, tag="wsb")
        nc.scalar.activation(scr[:], bones_sb[:], Exp, scale=1.0)
        # rc=0 projections fully before attention starts
        drain(proj_quanta(0))
        for i in range(2):
            nc.sync.dma_start(wo_hi[i][:], woh[i])
            nc.sync.dma_start(wo_lo[i][:], wol[i])

        ctx_hist: dict = {}

        # ---------------- attention ----------------
        for QT in range(QTN):
            ktmax = 4 * QT + 3 if causal else KTN - 1
            qcur = qp_tiles.pop(QT)
            cb = [None, None]
            cl = [None, None]
            for i in range(2):
                cb[i] = ctxp.tile([128, QTW], bf16, tag=f"cb{i}", name=f"cb{QT}_{i}")
                cl[i] = ctxp.tile([128, QTW], bf16, tag=f"cl{i}", name=f"cl{QT}_{i}")

            # filler: projections for QT+1; output projections deferred two
            # tiles so QT3's long ACT-bound sweep keeps the PE fed.
            fillers = [proj_quanta(QT + 1)] if QT + 1 < QTN else []
            if QT == QTN - 2 and QTN - 4 >= 0:
                fillers.append(outproj_quanta(QTN - 4, *ctx_hist.pop(QTN - 4)))
            if QT == QTN - 1:
                for j in (QTN - 3, QTN - 2):
                    if j >= 0:
                        fillers.append(outproj_quanta(j, *ctx_hist.pop(j)))
            bg = rr(*fillers) if fillers else None

            for i in range(2):  # head pair i: local heads 2i, 2i+1
                hA, hB = 2 * i, 2 * i + 1
                pcA = ps_ctx.tile([128, QTW], f32, tag="pctx", name=f"pc{QT}_{hA}")
                pcB = ps_ctx.tile([128, QTW], f32, tag="pctx", name=f"pc{QT}_{hB}")
                for kt in range(ktmax + 1):
                    diag = causal and kt >= 4 * QT
                    j = kt - 4 * QT if diag else 0
                    c0 = 128 * j
                    ps2 = ps_sT.tile([128, 2 * QTW], f32, tag="sT", name=f"sT{QT}_{i}_{kt}")
                    nc.tensor.matmul(
                        ps2[:, c0:QTW],
                        kp[i][0:D, 128 * kt : 128 * (kt + 1)],
                        qcur[i][0:D, c0:QTW],
                        start=True,
                        stop=True,
                    )
                    nc.tensor.matmul(
                        ps2[:, QTW + c0 : 2 * QTW],
                        kp[i][D:128, 128 * kt : 128 * (kt + 1)],
                        qcur[i][D:128, c0:QTW],
                        start=True,
                        stop=True,
                    )
                    # one exp for both heads (3D AP over the two banks)
                    pt2 = ptp.tile([128, 2 * QTW], f32r, tag="pt", name=f"pt{QT}_{i}_{kt}")
                    ps2v = ps2[:].rearrange("p (h q) -> p h q", h=2)
                    pt2v = pt2[:].rearrange("p (h q) -> p h q", h=2)
                    nc.scalar.activation(
                        pt2v[:, :, c0:QTW], ps2v[:, :, c0:QTW], Exp, scale=inv_sqrt_e
                    )
                    if diag:
                        for off in (0, QTW):
                            nc.vector.tensor_mul(
                                pt2[:, off + c0 : off + c0 + 128],
                                pt2[:, off + c0 : off + c0 + 128],
                                tri01_sb[:],
                            )
                    for pc, h, off in ((pcA, hA, 0), (pcB, hB, QTW)):
                        nc.tensor.matmul(
                            pc[:, c0:QTW],
                            v_aug[:, VSEG * kt + 128 * h : VSEG * kt + 128 * (h + 1)],
                            pt2[:, off + c0 : off + QTW],
                            start=(kt == 0),
                            stop=(kt == ktmax),
                            skip_group_check=True,
                        )
                    bg = pump(bg, 4)

                # ---- normalization for this pair
                srow2 = normp.tile([2, QTW], f32, tag="srow2", name=f"sr{QT}_{i}")
                nc.vector.tensor_copy(srow2[0:1, :], pcA[D : D + 1, :])
                stg = normp.tile([1, QTW], f32, tag="stg", name=f"stg{QT}_{i}")
                nc.vector.tensor_copy(stg[:], pcB[D : D + 1, :])
                nc.sync.dma_start(srow2[1:2, :], stg[:])
                srec2 = normp.tile([2, QTW], f32, tag="srec2")
                nc.vector.reciprocal_approx_fast(out=srec2[:], in_=srow2[:])
                srec2r = normp.tile([2, QTW], f32r, tag="srec2r")
                nc.vector.tensor_copy(srec2r[:], srec2[:])
                psb = ps_mm.tile([128, QTW], f32, tag="mm")
                nc.tensor.matmul(psb[:], bones_sb[:], srec2r[:], start=True, stop=True)
                bc = bcp.tile([128, QTW], f32, tag="bc", name=f"bc{QT}_{i}")
                nc.vector.tensor_copy(bc[:], psb[:])
                ctmp = normp.tile([128, QTW], f32, tag="ctmp", name=f"ct{QT}_{i}")
                for jj, pc in ((0, pcA), (1, pcB)):
                    sl = slice(D * jj, D * jj + D)
                    nc.vector.tensor_mul(ctmp[sl, :], pc[0:D, :], bc[sl, :])
                    nc.vector.tensor_copy(cb[i][sl, :], ctmp[sl, :])
                    nc.vector.tensor_sub(cl[i][sl, :], ctmp[sl, :], cb[i][sl, :])
                bg = pump(bg, 8)

            drain(bg)
            ctx_hist[QT] = (cb, cl)

        drain(outproj_quanta(QTN - 1, *ctx_hist.pop(QTN - 1)))


def _build(causal: bool):
    import concourse.mybir as mybir
    import concourse.tile as tile
    from concourse import bacc

    nc = bacc.Bacc("TRN2", target_bir_lowering=False, debug=False, num_devices=N_CORES)
    with tile.TileContext(nc) as tc:
        _emit(nc, tc, tile, mybir, causal)
    nc.compile()
    return nc


def _consts(causal: bool):
    bones = np.zeros((2, 128), dtype=np.float32)
    bones[0, 0:D] = 1.0
    bones[1, D:128] = 1.0
    consts = {
        "vones": np.ones((128, KTN * GH), dtype=np.float32),
        "bones2": bones,
    }
    if causal:
        f = np.arange(128)[None, :]
        p = np.arange(128)[:, None]
        consts["tri01"] = (f >= p).astype(np.float32)
    return consts


def _tile_x(xT):
    # (E, S) -> [ECN, QTN, 128, QTW] contiguous
    return np.ascontiguousarray(
        np.asarray(xT).reshape(ECN, 128, QTN, QTW).transpose(0, 2, 1, 3)
    )


def kernel(**inputs):
    import ml_dtypes
    import concourse.bass_utils as bass_utils

    bf = ml_dtypes.bfloat16
    key = np.asarray(inputs["key"], dtype=np.float32)
    query = np.asarray(inputs["query"], dtype=np.float32)
    value = np.asarray(inputs["value"], dtype=np.float32)
    Wk = np.asarray(inputs["Wk"], dtype=np.float32)
    Wq = np.asarray(inputs["Wq"], dtype=np.float32)
    Wv = np.asarray(inputs["Wv"], dtype=np.float32)
    Wo = np.asarray(inputs["Wo"], dtype=np.float32)
    causal = bool(np.asarray(inputs.get("mask", 1)).item())

    if causal not in _cache:
        _cache[causal] = _build(causal)
    nc = _cache[causal]
    consts = _consts(causal)

    in_maps = []
    for c in range(N_CORES):
        b, g = c // GROUPS, c % GROUPS
        gsl = slice(GD * g, GD * (g + 1))
        xvT = np.ascontiguousarray(value[b].T)         # (E, S)
        xvT_hi = xvT.astype(bf)
        xvT_lo = (xvT - xvT_hi.astype(np.float32)).astype(bf)
        wvT = np.ascontiguousarray(Wv[gsl, :].T)       # (E, GD)
        wvT_hi = wvT.astype(bf)
        wvT_lo = (wvT - wvT_hi.astype(np.float32)).astype(bf)
        woT = np.ascontiguousarray(Wo[:, gsl].T)       # (GD, E)
        woT_hi = woT.astype(bf)
        woT_lo = (woT - woT_hi.astype(np.float32)).astype(bf)
        m = {
            "xq4": _tile_x(query[b].T.astype(np.float16)),
            "xk4": _tile_x(key[b].T.astype(np.float16)),
            "xvh4": _tile_x(xvT_hi),
            "xvl4": _tile_x(xvT_lo),
            "wq4": np.ascontiguousarray(Wq[gsl, :].T.astype(np.float16)).reshape(ECN, 128, GD).copy(),
            "wk4": np.ascontiguousarray(Wk[gsl, :].T.astype(np.float16)).reshape(ECN, 128, GD).copy(),
            "wvh4": wvT_hi.reshape(ECN, 128, GD).copy(),
            "wvl4": wvT_lo.reshape(ECN, 128, GD).copy(),
            "woh": woT_hi.reshape(2, 128, E).copy(),
            "wol": woT_lo.reshape(2, 128, E).copy(),
        }
        m.update(consts)
        in_maps.append(m)

    res = kernel._last_results = bass_utils.run_bass_kernel_spmd(
        nc, in_maps, core_ids=list(range(N_CORES)), **kernel._run_kwargs
    )
    out = np.zeros((B, S, E), dtype=np.float32)
    for c in range(N_CORES):
        out[c // GROUPS] += res.results[c]["out"]
    return out


kernel._run_kwargs = {}
kernel._last_results = None


# revision 26
# speedup vs baseline: 1.0054x; 1.0054x over previous
"""Multi-head attention (B=2, S=2048, E=1024, H=16, causal) on 8 trn2 NeuronCores.

Sharding: 8 cores = 2 batches x 4 head-groups (4 heads / 256 embed dims per core).
Host sums the 4 partial output projections per batch and stacks batches.

v3 design:
  - Host pre-tiles every input into the exact [.., 128, W] tile layout so each
    dma_start is one contiguous DRAM read; DMAs are emitted in first-use order
    (wk -> xk[0] -> wq -> xq[0] -> ...) so the PE starts within a few us.
  - Scores via row-tiled K=64 matmul pairs: heads 2i/2i+1 packed on partitions
    0:64 / 64:128 of one kT/qT tile; the two matmuls occupy disjoint PE row
    groups and execute concurrently (~176ns per pair vs 227ns for one K=128
    matmul, HW-measured).
  - Causal mask: exp runs unmasked, then one DVE multiply zeroes the upper
    triangle of the 128-wide diagonal strip of P (replaces 64 mask matmuls).
  - One exp per (pair, kt): ACT reads the 2-bank PSUM pair tile with a 3D AP
    and writes both heads' P tiles in one call.
  - Precision: scores/AV in f32r (raw fp32 bits, PE rounds to ~2^-11); the
    value and output projections run as 3-term bf16 hi/lo splits
       v   = xv_h@wv_h + xv_l@wv_h + xv_h@wv_l     (all bf16, ~2^-17 effective)
       out = ctx_h@wo_h + ctx_l@wo_h + ctx_h@wo_l
    which beats f32r single-pass precision AND runs ~25% faster per matmul.
  - All copies on DVE (vector); ACT does exp only.
  - Projection work for chunk rc=QT+1 and the output projection of QT-1 are
    emitted round-robin in small quanta between attention kt-steps of QT so
    the PE never idles while ACT runs exp (keeps HAM warm).
"""

import sys

if "/opt/trn_rl_repo" not in sys.path:
    sys.path.insert(0, "/opt/trn_rl_repo")

import numpy as np

B = 2
S = 2048
E = 1024
H = 16
D = 64
N_CORES = 8
GROUPS = 4              # head-groups (cores per batch)
GH = H // GROUPS        # heads per core = 4
GD = GH * D             # qkv dims per core = 256
QTW = 512               # query-tile width
QTN = S // QTW          # 4
KTN = S // 128          # 16
ECN = E // 128          # embed chunks = 8
VSEG = GH * 128         # v_aug cols per kt = 512 (128-col aligned segments)

_cache: dict = {}


def _emit(nc, tc, tile, mybir, causal):
    f32 = mybir.dt.float32
    f32r = mybir.dt.float32r
    bf16 = mybir.dt.bfloat16
    f16 = mybir.dt.float16
    Exp = mybir.ActivationFunctionType.Exp
    inv_sqrt_e = 1.0 / float(np.sqrt(E))

    xq4 = nc.dram_tensor("xq4", [ECN, QTN, 128, QTW], f16, kind="ExternalInput").ap()
    xk4 = nc.dram_tensor("xk4", [ECN, QTN, 128, QTW], f16, kind="ExternalInput").ap()
    xvh4 = nc.dram_tensor("xvh4", [ECN, QTN, 128, QTW], bf16, kind="ExternalInput").ap()
    xvl4 = nc.dram_tensor("xvl4", [ECN, QTN, 128, QTW], bf16, kind="ExternalInput").ap()
    wq4 = nc.dram_tensor("wq4", [ECN, 128, GD], f16, kind="ExternalInput").ap()
    wk4 = nc.dram_tensor("wk4", [ECN, 128, GD], f16, kind="ExternalInput").ap()
    wvh4 = nc.dram_tensor("wvh4", [ECN, 128, GD], bf16, kind="ExternalInput").ap()
    wvl4 = nc.dram_tensor("wvl4", [ECN, 128, GD], bf16, kind="ExternalInput").ap()
    woh = nc.dram_tensor("woh", [2, 128, E], bf16, kind="ExternalInput").ap()
    wol = nc.dram_tensor("wol", [2, 128, E], bf16, kind="ExternalInput").ap()
    vones = nc.dram_tensor("vones", [128, KTN * GH], f32, kind="ExternalInput").ap()
    bones2 = nc.dram_tensor("bones2", [2, 128], f32r, kind="ExternalInput").ap()
    if causal:
        tri01 = nc.dram_tensor("tri01", [128, 128], f32, kind="ExternalInput").ap()
    out = nc.dram_tensor("out", [S, E], f32, kind="ExternalOutput").ap()
    dbg = nc.dram_tensor("dbg", [1, 8], f32, kind="ExternalOutput").ap()

    with (
        tc.tile_pool(name="xp", bufs=12) as xp,       # f32r x chunks (K/Q)
        tc.tile_pool(name="xhp", bufs=8) as xhp,      # xv hi chunks (bf16)
        tc.tile_pool(name="xlp", bufs=8) as xlp,      # xv lo chunks (bf16)
        tc.tile_pool(name="qpp", bufs=2) as qpp,      # per-QT q pair tiles
        tc.tile_pool(name="persist", bufs=1) as pp,
        tc.tile_pool(name="ptp", bufs=4) as ptp,
        tc.tile_pool(name="ctxp", bufs=3) as ctxp,    # per-QT ctx hi/lo (bf16)
        tc.tile_pool(name="normp", bufs=2) as normp,
        tc.tile_pool(name="bcp", bufs=2) as bcp,
        tc.tile_pool(name="ostp", bufs=3) as ostp,
        tc.tile_pool(name="ps_mm", bufs=2, space="PSUM") as ps_mm,
        tc.tile_pool(name="ps_sT", bufs=2, space="PSUM") as ps_sT,
        tc.tile_pool(name="ps_ctx", bufs=2, space="PSUM") as ps_ctx,
    ):
        # ---------------- persistent tiles ----------------
        kp = [pp.tile([128, S], f32r, tag=f"kp{i}", name=f"kp{i}") for i in range(2)]
        v_aug = pp.tile([128, KTN * VSEG], f32r, tag="v_aug")
        wo_hi = [pp.tile([128, E], bf16, tag=f"woh{i}", name=f"woh{i}") for i in range(2)]
        wo_lo = [pp.tile([128, E], bf16, tag=f"wol{i}", name=f"wol{i}") for i in range(2)]
        wk = [pp.tile([128, GD], f16, tag=f"wk{i}", name=f"wk{i}") for i in range(ECN)]
        wq = [pp.tile([128, GD], f16, tag=f"wq{i}", name=f"wq{i}") for i in range(ECN)]
        wvh = [pp.tile([128, GD], bf16, tag=f"wvh{i}", name=f"wvh{i}") for i in range(ECN)]
        wvl = [pp.tile([128, GD], bf16, tag=f"wvl{i}", name=f"wvl{i}") for i in range(ECN)]
        bones_sb = pp.tile([2, 128], f32r, tag="bones_sb")
        vones_sb = pp.tile([128, KTN * GH], f32, tag="vones_sb")
        if causal:
            tri01_sb = pp.tile([128, 128], f32, tag="tri01_sb")

        qp_tiles: dict = {}

        # Dummy warmup accumulation: keeps the PE HAM clock at 8/8 through the
        # DMA-bound first chunk (uses a score-pool PSUM slot, idle until QT0).
        warm = {"ps": None, "n": 0}

        def emit_warm(k=2):
            if warm["ps"] is None:
                warm["ps"] = ps_sT.tile([128, 2 * QTW], f32, tag="sT", name="warm")
            for _ in range(k):
                nc.tensor.matmul(
                    warm["ps"][:, 0:GD],
                    wk[0][:, 0:128],
                    wk[0][:],
                    start=(warm["n"] == 0),
                    stop=False,
                    skip_group_check=True,
                )
                warm["n"] += 1

        def finish_warm():
            if warm["ps"] is None:
                return
            nc.tensor.matmul(
                warm["ps"][:, 0:GD],
                wk[0][:, 0:128],
                wk[0][:],
                start=False,
                stop=True,
                skip_group_check=True,
            )
            wsb = normp.tile([1, 8], f32, tag="wsb")
            nc.vector.tensor_copy(wsb[:], warm["ps"][0:1, 0:8])
            nc.sync.dma_start(dbg[:], wsb[:])
            warm["ps"] = None

        def with_warm(gen):
            for x in gen:
                emit_warm(2)
                yield x

        # ---------------- projection emission (quantum generator) -----
        def proj_quanta(rc):
            r_sl = slice(QTW * rc, QTW * (rc + 1))
            qcur = [
                qpp.tile([128, QTW], f32r, tag=f"qp{i}", name=f"qp{rc}_{i}")
                for i in range(2)
            ]
            qp_tiles[rc] = qcur
            for which, xsrc, wts, w4 in (("k", xk4, wk, wk4), ("q", xq4, wq, wq4)):
                if rc == 0:
                    for ec in range(ECN):
                        nc.sync.dma_start(wts[ec][:], w4[ec])
                xsl = []
                for ec in range(ECN):
                    t = xp.tile([128, QTW], f16, tag="x", name=f"x{which}{rc}_{ec}")
                    nc.sync.dma_start(t[:], xsrc[ec, rc])
                    xsl.append(t)
                    if ec % 4 == 3:
                        yield
                for dt_ in range(2):
                    ps = ps_mm.tile([128, QTW], f32, tag="mm")
                    for ec in range(ECN):
                        nc.tensor.matmul(
                            ps[:],
                            wts[ec][:, 128 * dt_ : 128 * (dt_ + 1)],
                            xsl[ec][:],
                            start=(ec == 0),
                            stop=(ec == ECN - 1),
                        )
                        if ec % 4 == 3:
                            yield
                    if which == "k":
                        nc.vector.tensor_copy(kp[dt_][:, r_sl], ps[:])
                    else:
                        nc.vector.tensor_copy(qcur[dt_][:], ps[:])
                    yield
            # V: 3-term bf16 projection from host-split hi/lo
            if rc == 0:
                for ec in range(ECN):
                    nc.sync.dma_start(wvh[ec][:], wvh4[ec])
                    nc.sync.dma_start(wvl[ec][:], wvl4[ec])
            xh, xl = [], []
            for ec in range(ECN):
                th = xhp.tile([128, QTW], bf16, tag="xh", name=f"xvh{rc}_{ec}")
                tl = xlp.tile([128, QTW], bf16, tag="xl", name=f"xvl{rc}_{ec}")
                nc.sync.dma_start(th[:], xvh4[ec, rc])
                nc.sync.dma_start(tl[:], xvl4[ec, rc])
                xh.append(th)
                xl.append(tl)
                if ec % 4 == 3:
                    yield
            if rc == 0:
                # consts needed by attention QT=0 (emitted after first-use DMAs)
                nc.sync.dma_start(vones_sb[:], vones[:])
                if causal:
                    nc.sync.dma_start(tri01_sb[:], tri01[:])
                vh_view = v_aug[:].rearrange("p (k h d) -> p k h d", k=KTN, h=GH)
                nc.vector.tensor_copy(
                    vh_view[:, :, :, D],
                    vones_sb[:].rearrange("p (k h) -> p k h", k=KTN),
                )
            for rt in range(4):
                kt = rc * 4 + rt
                c_sl = slice(128 * rt, 128 * (rt + 1))
                ps = ps_mm.tile([128, GD], f32, tag="mm")
                n3 = 3 * ECN
                i3 = 0
                for ec in range(ECN):
                    for sta, mov in ((xh[ec], wvh[ec]), (xl[ec], wvh[ec]), (xh[ec], wvl[ec])):
                        nc.tensor.matmul(
                            ps[:],
                            sta[:, c_sl],
                            mov[:],
                            start=(i3 == 0),
                            stop=(i3 == n3 - 1),
                        )
                        i3 += 1
                    if ec % 2 == 1:
                        yield
                for h in range(GH):
                    nc.vector.tensor_copy(
                        v_aug[:, VSEG * kt + 128 * h : VSEG * kt + 128 * h + D],
                        ps[:, D * h : D * (h + 1)],
                    )
                yield

        # ---------------- output projection (quantum generator) -------
        def outproj_quanta(QT, cb, cl):
            for rt in range(4):
                r0 = QTW * QT + 128 * rt
                c_sl = slice(128 * rt, 128 * (rt + 1))
                ost = ostp.tile([128, E], f32, tag="ost")
                for ct in range(2):
                    po_ = ps_mm.tile([128, QTW], f32, tag="mm")
                    w_sl = slice(QTW * ct, QTW * (ct + 1))
                    i6 = 0
                    for i in range(2):
                        for sta, mov in (
                            (cb[i], wo_hi[i]),
                            (cl[i], wo_hi[i]),
                            (cb[i], wo_lo[i]),
                        ):
                            nc.tensor.matmul(
                                po_[:],
                                sta[:, c_sl],
                                mov[:, w_sl],
                                start=(i6 == 0),
                                stop=(i6 == 5),
                            )
                            i6 += 1
                        yield
                    nc.vector.tensor_copy(ost[:, w_sl], po_[:])
                nc.sync.dma_start(out[r0 : r0 + 128, :], ost[:])
                yield

        def drain(gen):
            if gen is not None:
                for _ in gen:
                    pass

        def pump(gen, n=1):
            if gen is None:
                return None
            try:
                for _ in range(n):
                    next(gen)
                return gen
            except StopIteration:
                return None

        def rr(*gens):
            """Round-robin interleave of generators."""
            gens = [g for g in gens if g is not None]
            while gens:
                for g in list(gens):
                    try:
                        yield next(g)
                    except StopIteration:
                        gens.remove(g)

        # preload the ACT exp table set (~2.7us) before it's needed at QT0:
        # a dummy exp on the first-loaded const forces the PSEUDO_LOAD early.
        nc.sync.dma_start(bones_sb[:], bones2[:])
        scr = normp.tile([2, 128], f32, tag="wsb")
        nc.scalar.activation(scr[:], bones_sb[:], Exp, scale=1.0)
        # rc=0 projections fully before attention starts
        drain(proj_quanta(0))
        for i in range(2):
            nc.sync.dma_start(wo_hi[i][:], woh[i])
            nc.sync.dma_start(wo_lo[i][:], wol[i])

        ctx_hist: dict = {}

        # ---------------- attention ----------------
        for QT in range(QTN):
            ktmax = 4 * QT + 3 if causal else KTN - 1
            qcur = qp_tiles.pop(QT)
            cb = [None, None]
            cl = [None, None]
            for i in range(2):
                cb[i] = ctxp.tile([128, QTW], bf16, tag=f"cb{i}", name=f"cb{QT}_{i}")
                cl[i] = ctxp.tile([128, QTW], bf16, tag=f"cl{i}", name=f"cl{QT}_{i}")

            # filler: projections for QT+1; output projections deferred two
            # tiles so QT3's long ACT-bound sweep keeps the PE fed.
            fillers = [proj_quanta(QT + 1)] if QT + 1 < QTN else []
            if QT == QTN - 2 and QTN - 4 >= 0:
                fillers.append(outproj_quanta(QTN - 4, *ctx_hist.pop(QTN - 4)))
            if QT == QTN - 1:
                for j in (QTN - 3, QTN - 2):
                    if j >= 0:
                        fillers.append(outproj_quanta(j, *ctx_hist.pop(j)))
            bg = rr(*fillers) if fillers else None

            for i in range(2):  # head pair i: local heads 2i, 2i+1
                hA, hB = 2 * i, 2 * i + 1
                pcA = ps_ctx.tile([128, QTW], f32, tag="pctx", name=f"pc{QT}_{hA}")
                pcB = ps_ctx.tile([128, QTW], f32, tag="pctx", name=f"pc{QT}_{hB}")
                for kt in range(ktmax + 1):
                    diag = causal and kt >= 4 * QT
                    j = kt - 4 * QT if diag else 0
                    c0 = 128 * j
                    ps2 = ps_sT.tile([128, 2 * QTW], f32, tag="sT", name=f"sT{QT}_{i}_{kt}")
                    nc.tensor.matmul(
                        ps2[:, c0:QTW],
                        kp[i][0:D, 128 * kt : 128 * (kt + 1)],
                        qcur[i][0:D, c0:QTW],
                        start=True,
                        stop=True,
                    )
                    nc.tensor.matmul(
                        ps2[:, QTW + c0 : 2 * QTW],
                        kp[i][D:128, 128 * kt : 128 * (kt + 1)],
                        qcur[i][D:128, c0:QTW],
                        start=True,
                        stop=True,
                    )
                    # one exp for both heads (3D AP over the two banks)
                    pt2 = ptp.tile([128, 2 * QTW], f32r, tag="pt", name=f"pt{QT}_{i}_{kt}")
                    ps2v = ps2[:].rearrange("p (h q) -> p h q", h=2)
                    pt2v = pt2[:].rearrange("p (h q) -> p h q", h=2)
                    nc.scalar.activation(
                        pt2v[:, :, c0:QTW], ps2v[:, :, c0:QTW], Exp, scale=inv_sqrt_e
                    )
                    if diag:
                        for off in (0, QTW):
                            nc.vector.tensor_mul(
                                pt2[:, off + c0 : off + c0 + 128],
                                pt2[:, off + c0 : off + c0 + 128],
                                tri01_sb[:],
                            )
                    for pc, h, off in ((pcA, hA, 0), (pcB, hB, QTW)):
                        nc.tensor.matmul(
                            pc[:, c0:QTW],
                            v_aug[:, VSEG * kt + 128 * h : VSEG * kt + 128 * (h + 1)],
                            pt2[:, off + c0 : off + QTW],
                            start=(kt == 0),
                            stop=(kt == ktmax),
                            skip_group_check=True,
                        )
                    bg = pump(bg, 4)

                # ---- normalization for this pair
                srow2 = normp.tile([2, QTW], f32, tag="srow2", name=f"sr{QT}_{i}")
                nc.vector.tensor_copy(srow2[0:1, :], pcA[D : D + 1, :])
                stg = normp.tile([1, QTW], f32, tag="stg", name=f"stg{QT}_{i}")
                nc.vector.tensor_copy(stg[:], pcB[D : D + 1, :])
                nc.sync.dma_start(srow2[1:2, :], stg[:])
                srec2 = normp.tile([2, QTW], f32, tag="srec2")
                nc.vector.reciprocal_approx_fast(out=srec2[:], in_=srow2[:])
                srec2r = normp.tile([2, QTW], f32r, tag="srec2r")
                nc.vector.tensor_copy(srec2r[:], srec2[:])
                psb = ps_mm.tile([128, QTW], f32, tag="mm")
                nc.tensor.matmul(psb[:], bones_sb[:], srec2r[:], start=True, stop=True)
                bc = bcp.tile([128, QTW], f32, tag="bc", name=f"bc{QT}_{i}")
                nc.vector.tensor_copy(bc[:], psb[:])
                ctmp = normp.tile([128, QTW], f32, tag="ctmp", name=f"ct{QT}_{i}")
                for jj, pc in ((0, pcA), (1, pcB)):
                    sl = slice(D * jj, D * jj + D)
                    nc.vector.tensor_mul(ctmp[sl, :], pc[0:D, :], bc[sl, :])
                    nc.vector.tensor_copy(cb[i][sl, :], ctmp[sl, :])
                    nc.vector.tensor_sub(cl[i][sl, :], ctmp[sl, :], cb[i][sl, :])
                bg = pump(bg, 8)

            drain(bg)
            ctx_hist[QT] = (cb, cl)

        drain(outproj_quanta(QTN - 1, *ctx_hist.pop(QTN - 1)))


def _build(causal: bool):
    import concourse.mybir as mybir
    import concourse.tile as tile
    from concourse import bacc

    nc = bacc.Bacc("TRN2", target_bir_lowering=False, debug=False, num_devices=N_CORES)
    with tile.TileContext(nc) as tc:
        _emit(nc, tc, tile, mybir, causal)
    nc.compile()
    return nc


def _consts(causal: bool):
    bones = np.zeros((2, 128), dtype=np.float32)
    bones[0, 0:D] = 1.0
    bones[1, D:128] = 1.0
    consts = {
        "vones": np.ones((128, KTN * GH), dtype=np.float32),
        "bones2": bones,
    }
    if causal:
        f = np.arange(128)[None, :]
        p = np.arange(128)[:, None]
        consts["tri01"] = (f >= p).astype(np.float32)
    return consts


def _tile_x(xT):
    # (E, S) -> [ECN, QTN, 128, QTW] contiguous
    return np.ascontiguousarray(
        np.asarray(xT).reshape(ECN, 128, QTN, QTW).transpose(0, 2, 1, 3)
    )


def kernel(**inputs):
    import ml_dtypes
    import concourse.bass_utils as bass_utils

    bf = ml_dtypes.bfloat16
    key = np.asarray(inputs["key"], dtype=np.float32)
    query = np.asarray(inputs["query"], dtype=np.float32)
    value = np.asarray(inputs["value"], dtype=np.float32)
    Wk = np.asarray(inputs["Wk"], dtype=np.float32)
    Wq = np.asarray(inputs["Wq"], dtype=np.float32)
    Wv = np.asarray(inputs["Wv"], dtype=np.float32)
    Wo = np.asarray(inputs["Wo"], dtype=np.float32)
    causal = bool(np.asarray(inputs.get("mask", 1)).item())

    if causal not in _cache:
        _cache[causal] = _build(causal)
    nc = _cache[causal]
    consts = _consts(causal)

    in_maps = []
    for c in range(N_CORES):
        b, g = c // GROUPS, c % GROUPS
        gsl = slice(GD * g, GD * (g + 1))
        xvT = np.ascontiguousarray(value[b].T)         # (E, S)
        xvT_hi = xvT.astype(bf)
        xvT_lo = (xvT - xvT_hi.astype(np.float32)).astype(bf)
        wvT = np.ascontiguousarray(Wv[gsl, :].T)       # (E, GD)
        wvT_hi = wvT.astype(bf)
        wvT_lo = (wvT - wvT_hi.astype(np.float32)).astype(bf)
        woT = np.ascontiguousarray(Wo[:, gsl].T)       # (GD, E)
        woT_hi = woT.astype(bf)
        woT_lo = (woT - woT_hi.astype(np.float32)).astype(bf)
        m = {
            "xq4": _tile_x(query[b].T.astype(np.float16)),
            "xk4": _tile_x(key[b].T.astype(np.float16)),
            "xvh4": _tile_x(xvT_hi),
            "xvl4": _tile_x(xvT_lo),
            "wq4": np.ascontiguousarray(Wq[gsl, :].T.astype(np.float16)).reshape(ECN, 128, GD).copy(),
            "wk4": np.ascontiguousarray(Wk[gsl, :].T.astype(np.float16)).reshape(ECN, 128, GD).copy(),
            "wvh4": wvT_hi.reshape(ECN, 128, GD).copy(),
            "wvl4": wvT_lo.reshape(ECN, 128, GD).copy(),
            "woh": woT_hi.reshape(2, 128, E).copy(),
            "wol": woT_lo.reshape(2, 128, E).copy(),
        }
        m.update(consts)
        in_maps.append(m)

    res = kernel._last_results = bass_utils.run_bass_kernel_spmd(
        nc, in_maps, core_ids=list(range(N_CORES)), **kernel._run_kwargs
    )
    out = np.zeros((B, S, E), dtype=np.float32)
    for c in range(N_CORES):
        out[c // GROUPS] += res.results[c]["out"]
    return out


kernel._run_kwargs = {}
kernel._last_results = None
